# revision 31
# baseline (speedup 1.0000x reference)
"""Trainium2 Bass kernel for nn_ColbertAdapter (ColBERT late-interaction adapter).

Strategy (8 NeuronCores, single SPMD launch):
  - Context index (C=2048 entries) is sharded 256/core; queries replicated.
  - Inputs uploaded bf16; LN weights + 1/sqrt(dk) folded into projection
    weights on host. LN normalize runs on the scalar engine (Identity with
    per-partition scale/bias), stats on vector.
  - PE p-state aware scheduling: the tensor engine only reaches 2.4 GHz
    after ~3us of gapless execution, so LN/transpose/projection emission is
    interleaved to keep the PE dense, score PSUM tiles are single-bank
    [128,512] with a 4-deep ring, and every PSUM consumer is split across
    DVE/ACT so the PE never waits.
  - Scores computed transposed S^T[cu, t]; MaxSim over U as bf16 chained
    maxes straight out of PSUM, one Exp per column group into bf16.
  - attn@v via PE with a ones-column augmenting v so the softmax denominator
    falls out of the same matmul. Softmax uses a fixed zero max-offset:
    LN output norm is exactly sqrt(D), so |logits| stays well below 87
    (checked on host via power iteration) and exp cannot overflow.
  - Per head-pair bf16 ReduceScatter merges partial numerators+denominators;
    a tiny warmup collective right after launch absorbs the first-collective
    setup latency; pair epilogues (divide -> wo accumulate) are pipelined
    under later heads' compute; only LN4 -> wp runs after the last RS.
"""

import os
import sys

try:
    import concourse  # noqa: F401
except ImportError:
    for p in ("/opt/trn_rl_repo", "/root/.axon_site/_ro/trn_rl_repo"):
        if os.path.isdir(p):
            sys.path.insert(0, p)
            break

import numpy as np
import ml_dtypes

import concourse.bass as bass
import concourse.mybir as mybir
from concourse import tile, bacc, bass_utils
from concourse.alu_op_type import AluOpType

BF16 = mybir.dt.bfloat16
F32 = mybir.dt.float32

NCORES = 8
B, T, C, U, D, P = 4, 256, 2048, 4, 512, 512
H = 8
DK = D // H
BT = B * T              # 1024 query tokens
CS = C // NCORES        # 256 contexts per core
CUS = CS * U            # 1024 key rows per core
TSH = BT // NCORES      # 128 tokens per core in the output shard
EPS = 1e-5

_CACHE = {}


def build_nc():
    nc = bacc.Bacc("TRN2", target_bir_lowering=False, debug=False,
                   num_devices=NCORES)

    # ---- DRAM I/O ----
    x_d = nc.dram_tensor("x", [BT, D], BF16, kind="ExternalInput").ap()
    kin_d = nc.dram_tensor("kin", [CUS, D], BF16, kind="ExternalInput").ap()
    vin_d = nc.dram_tensor("vin", [CS, D], BF16, kind="ExternalInput").ap()
    w_d = {
        n: nc.dram_tensor(n, [D, D], BF16, kind="ExternalInput").ap()
        for n in ("wq", "wk", "wv", "wo", "wp")
    }
    bq_d = nc.dram_tensor("bq", [D], F32, kind="ExternalInput").ap()
    bk_d = nc.dram_tensor("bk", [D], F32, kind="ExternalInput").ap()
    bv_d = nc.dram_tensor("bv", [D], F32, kind="ExternalInput").ap()
    bo_d = nc.dram_tensor("bo", [D], BF16, kind="ExternalInput").ap()
    ind_d = nc.dram_tensor("ind", [2, 128], BF16, kind="ExternalInput").ap()
    eye_d = nc.dram_tensor("eye", [128, 128], BF16, kind="ExternalInput").ap()
    bp_d = nc.dram_tensor("bp", [D], BF16, kind="ExternalInput").ap()
    y_d = nc.dram_tensor("y", [TSH, P], BF16, kind="ExternalOutput").ap()

    with tile.TileContext(nc) as tc:
        from contextlib import ExitStack
        ctx = ExitStack()
        with ctx:
            persist = ctx.enter_context(tc.tile_pool(name="persist", bufs=1))
            small = ctx.enter_context(tc.tile_pool(name="small", bufs=4))
            lnin = ctx.enter_context(tc.tile_pool(name="lnin", bufs=4))
            lnout = ctx.enter_context(tc.tile_pool(name="lnout", bufs=4))
            pall = ctx.enter_context(tc.tile_pool(name="pall", bufs=4))
            pmax = ctx.enter_context(tc.tile_pool(name="pmax", bufs=6))
            o65p = ctx.enter_context(tc.tile_pool(name="o65", bufs=4))
            psum = ctx.enter_context(
                tc.tile_pool(name="psum", bufs=3, space="PSUM"))
            dram = ctx.enter_context(
                tc.tile_pool(name="dram", bufs=1, space="DRAM"))

            # ---- small constants first (transposes need eye) ----
            eye = persist.tile([128, 128], BF16, tag="eye")
            nc.sync.dma_start(eye[:], eye_d)
            ind = persist.tile([2, 128], BF16, tag="ind")
            nc.sync.dma_start(ind[:], ind_d)

            # tiny warmup collective: pays the first-collective setup cost
            # while LN/projections run, so the real RS0 starts promptly
            warm_sb = persist.tile([8, 64], BF16, tag="warm")
            nc.vector.memset(warm_sb[:], 0.0)
            warm_in = dram.tile([8, 64], BF16, name="warm_in")
            warm_out = dram.tile([1, 64], BF16, name="warm_out")
            nc.sync.dma_start(warm_in[:], warm_sb[:])
            nc.gpsimd.collective_compute(
                "ReduceScatter", AluOpType.add,
                replica_groups=[list(range(NCORES))],
                ins=[warm_in.opt()], outs=[warm_out.opt()],
            )

            # ---- LayerNorm helper: stats on DVE, normalize on ACT ----
            def emit_ln(x_tile, out_tile):
                stats6 = small.tile([128, 6], F32, tag="bns")
                nc.vector.bn_stats(stats6[:], x_tile[:])
                mv = small.tile([128, 2], F32, tag="bna")
                nc.vector.bn_aggr(mv[:], stats6[:])
                veps = small.tile([128, 1], F32, tag="veps")
                nc.vector.tensor_scalar_add(veps[:], mv[:, 1:2], EPS)
                std = small.tile([128, 1], F32, tag="std")
                nc.scalar.sqrt(std[:], veps[:])
                rstd = small.tile([128, 1], F32, tag="rstd")
                nc.vector.reciprocal(rstd[:], std[:])
                negmr = small.tile([128, 1], F32, tag="negmr")
                nc.vector.tensor_scalar(
                    negmr[:], mv[:, 0:1], rstd[:], -1.0,
                    op0=AluOpType.mult, op1=AluOpType.mult)
                nc.scalar.activation(
                    out_tile[:], x_tile[:],
                    mybir.ActivationFunctionType.Identity,
                    bias=negmr[:, 0:1], scale=rstd[:, 0:1])

            evac_i = [0]

            def evac_engine():
                evac_i[0] += 1
                return nc.vector if evac_i[0] % 2 == 0 else nc.scalar

            # ---- LN + transpose staging ----
            def stage(src_ap, dstT, tiles):
                # dstT: [128, 4, n_tiles, 128] bf16 == srcLN^T blocks
                for i in tiles:
                    xt = lnin.tile([128, D], BF16, tag="lnin")
                    nc.sync.dma_start(xt[:], src_ap[i * 128:(i + 1) * 128, :])
                    lt = lnout.tile([128, D], BF16, tag="lnout")
                    emit_ln(xt, lt)
                    tp = psum.tile([128, 512], BF16, tag="wide")
                    for jb in range(4):
                        nc.tensor.transpose(
                            tp[:, jb * 128:(jb + 1) * 128],
                            lt[:, jb * 128:(jb + 1) * 128], eye[:])
                    eng = evac_engine()
                    if eng is nc.vector:
                        nc.vector.tensor_copy(
                            dstT[:, :, i, :],
                            tp[:].rearrange("p (b t) -> p b t", b=4))
                    else:
                        nc.scalar.copy(
                            dstT[:, :, i, :],
                            tp[:].rearrange("p (b t) -> p b t", b=4))

            def proj_chunk(dstT, wname, srcT, bias, tch):
                # dstT[:, jt, tch*512:(tch+1)*512] over all 4 jt blocks
                for jt in range(4):
                    ps = psum.tile([128, 512], F32, tag="wide",
                                   name=f"ps_{wname}_{jt}_{tch}")
                    for dt in range(4):
                        nc.tensor.matmul(
                            ps[:],
                            lhsT=w_sb[wname][:, dt,
                                             jt * 128:(jt + 1) * 128],
                            rhs=srcT[:, dt, tch * 4:(tch + 1) * 4, :],
                            start=(dt == 0), stop=(dt == 3))
                    eng = evac_engine()
                    dst = dstT[:, jt, tch * 512:(tch + 1) * 512]
                    if eng is nc.vector:
                        nc.vector.tensor_scalar_add(
                            dst, ps[:], bias[:, jt:jt + 1])
                    else:
                        nc.scalar.add(dst, ps[:], bias[:, jt:jt + 1])

            # weights / biases: issue DMAs interleaved with LN input DMAs so
            # the first projection's weights arrive just in time
            w_sb = {}
            for n in ("wq", "wk", "wv", "wo", "wp"):
                w_sb[n] = persist.tile([128, 4, D], BF16, tag=f"w_{n}",
                                       name=f"w_{n}")

            def load_w(n):
                nc.sync.dma_start(
                    w_sb[n][:], w_d[n].rearrange("(b p) j -> p b j", p=128))

            bq_sb = persist.tile([128, 4], F32, tag="bq")
            bk_sb = persist.tile([128, 4], F32, tag="bk")
            bv_sb = persist.tile([128, 4], F32, tag="bv")

            xnT = persist.tile([128, 4, 8, 128], BF16, tag="xnT")
            knT = persist.tile([128, 4, 8, 128], BF16, tag="knT")
            vnT = persist.tile([128, 4, 2, 128], BF16, tag="vnT")
            qT = persist.tile([128, 4, BT], BF16, tag="qT")
            kT = persist.tile([128, 4, CUS], BF16, tag="kT")

            stage(x_d, xnT, [0, 1, 2, 3])
            load_w("wq")
            nc.sync.dma_start(bq_sb[:], bq_d.rearrange("(b p) -> p b", p=128))
            stage(x_d, xnT, [4, 5, 6, 7])
            proj_chunk(qT, "wq", xnT, bq_sb, 0)
            stage(kin_d, knT, [0, 1, 2, 3])
            load_w("wk")
            nc.sync.dma_start(bk_sb[:], bk_d.rearrange("(b p) -> p b", p=128))
            proj_chunk(qT, "wq", xnT, bq_sb, 1)
            stage(kin_d, knT, [4, 5, 6, 7])
            proj_chunk(kT, "wk", knT, bk_sb, 0)
            stage(vin_d, vnT, [0, 1])
            load_w("wv")
            nc.sync.dma_start(bv_sb[:], bv_d.rearrange("(b p) -> p b", p=128))
            proj_chunk(kT, "wk", knT, bk_sb, 1)
            load_w("wo")
            load_w("wp")
            bo_row = persist.tile([1, D], BF16, tag="bo_row")
            nc.sync.dma_start(bo_row[:], bo_d.rearrange("(o d) -> o d", o=1))
            bp_row = persist.tile([1, D], BF16, tag="bp_row")
            nc.sync.dma_start(bp_row[:], bp_d.rearrange("(o d) -> o d", o=1))
            ones_row = persist.tile([1, 128], BF16, tag="ones_row")
            nc.vector.memset(ones_row[:], 1.0)

            # v[c, hd] with ones column per head -> v_sb[ct]: [128, 8, 65]
            # (bv is added after the attention average: attn weights sum to 1)
            v_sb = []
            for ct in range(2):
                vt = persist.tile([128, 8, 65], BF16, tag=f"v_sb{ct}",
                                  name=f"v_sb{ct}")
                ps = psum.tile([128, 512], F32, tag="wide")
                for dt in range(4):
                    nc.tensor.matmul(
                        ps[:],
                        lhsT=vnT[:, dt, ct, :],
                        rhs=w_sb["wv"][:, dt, :],
                        start=(dt == 0), stop=(dt == 3))
                nc.vector.tensor_copy(
                    vt[:, :, 0:64],
                    ps[:].rearrange("p (h e) -> p h e", h=8))
                nc.vector.memset(vt[:, :, 64:65], 1.0)
                v_sb.append(vt)

            # ---- per-head: scores^T -> U-max -> exp -> attn@v_aug ----
            bounce_ins = [
                dram.tile([NCORES, 130, TSH], BF16, name=f"bin{i}")
                for i in range(4)
            ]
            bounce_outs = [
                dram.tile([130, TSH], BF16, name=f"bout{i}")
                for i in range(4)
            ]
            bviews = [bo_.rearrange("(h j) t -> h j t", j=65)
                      for bo_ in bounce_outs]
            ob = persist.tile([128, 4, TSH], BF16, tag="ob")
            s_sb = persist.tile([2, 4, TSH], BF16, tag="s_sb")
            o_n = persist.tile([128, 4, TSH], BF16, tag="o_n")
            psy = psum.tile([128, 512], F32, tag="psy", bufs=1, name="psy")

            def emit_pair_epilogue(p):
                # readback merged o^T (+denominators) for our token shard
                nc.sync.dma_start(ob[0:64, p, :], bviews[p][0, 0:64, :])
                nc.sync.dma_start(ob[64:128, p, :], bviews[p][1, 0:64, :])
                nc.sync.dma_start(s_sb[:, p, :], bviews[p][:, 64, :])
                # broadcast the 2 denominator rows to the pair's 128 o^T rows
                psd = psum.tile([128, TSH], F32, tag="attnv", bufs=1,
                                name=f"psd{p}")
                nc.tensor.matmul(psd[:], lhsT=ind[:],
                                 rhs=s_sb[:, p, :],
                                 start=True, stop=True)
                rb = small.tile([128, TSH], F32, tag="rb")
                nc.vector.reciprocal(rb[:], psd[:])
                t = small.tile([128, TSH], F32, tag="odiv")
                nc.vector.tensor_mul(t[:], ob[:, p, :], rb[:])
                nc.vector.tensor_scalar_add(
                    o_n[:, p, :], t[:], bv_sb[:, p:p + 1])
                nc.tensor.matmul(psy[:], lhsT=o_n[:, p, :],
                                 rhs=w_sb["wo"][:, p, :],
                                 start=(p == 0), stop=False)

            def emit_attnv(h, pm):
                # attn@v for head h (software-pipelined one head behind the
                # score matmuls so the PE never waits on the exp chain)
                o65 = o65p.tile([65, 1024], BF16, tag="o65")
                for tch in range(2):
                    pso = psum.tile([65, 512], F32, tag="attnv", bufs=1,
                                    name=f"pso{h}_{tch}")
                    for c2 in range(2):
                        nc.tensor.matmul(
                            pso[:],
                            lhsT=v_sb[c2][:, h, :],
                            rhs=pm[c2][:, tch * 512:(tch + 1) * 512],
                            start=(c2 == 0), stop=(c2 == 1))
                    eng = nc.scalar if tch == 0 else nc.vector
                    if eng is nc.scalar:
                        nc.scalar.copy(
                            o65[:, tch * 512:(tch + 1) * 512], pso[:])
                    else:
                        nc.vector.tensor_copy(
                            o65[:, tch * 512:(tch + 1) * 512], pso[:])
                b_in = bounce_ins[h // 2]
                hh = h % 2
                nc.sync.dma_start(
                    b_in[:, hh * 65:(hh + 1) * 65, :].rearrange(
                        "s r t -> r s t"),
                    o65.rearrange("r (s t) -> r s t", s=NCORES))
                if h % 2 == 1:
                    nc.gpsimd.collective_compute(
                        "ReduceScatter", AluOpType.add,
                        replica_groups=[list(range(NCORES))],
                        ins=[bounce_ins[h // 2].rearrange("s r t -> (s r) t")],
                        outs=[bounce_outs[h // 2].opt()],
                    )

            pm_prev = None
            for h in range(H):
                hp = (h % 2) * 64
                jt = h // 2
                pm = []
                for c2 in range(2):
                    # 5-copy/3-mixed split: ACT evacuates most PSUM tiles to
                    # bf16; DVE chains maxes (one PSUM operand max each)
                    ncopy = 3 if c2 == 0 else 2
                    m = None
                    pss = []
                    for u in range(4):
                        r = 2 * u + c2
                        ps = psum.tile([128, 1024], F32, tag="wide",
                                       name=f"ps_h{h}_r{r}")
                        for tch in range(2):
                            nc.tensor.matmul(
                                ps[:, tch * 512:(tch + 1) * 512],
                                lhsT=kT[hp:hp + 64, jt,
                                        r * 128:(r + 1) * 128],
                                rhs=qT[hp:hp + 64, jt,
                                       tch * 512:(tch + 1) * 512],
                                start=True, stop=True)
                        last = (u == 3)
                        dst = None
                        if u < ncopy:
                            s = pmax.tile([128, 1024], BF16, tag="pm")
                            nc.scalar.copy(s[:], ps[:])
                            if m is None:
                                m = s
                            else:
                                m2 = (pall.tile([128, 1024], BF16, tag="pmf",
                                                name=f"pmf{h}_{c2}")
                                      if last else
                                      pmax.tile([128, 1024], BF16, tag="pm"))
                                nc.vector.tensor_max(m2[:], s[:], m[:])
                                m = m2
                        else:
                            m2 = (pall.tile([128, 1024], BF16, tag="pmf",
                                            name=f"pmf{h}_{c2}")
                                  if last else
                                  pmax.tile([128, 1024], BF16, tag="pm"))
                            nc.vector.tensor_max(m2[:], ps[:], m[:])
                            m = m2
                        # software-pipelined attnv of the previous head goes
                        # right after the first column group's matmuls
                        if c2 == 0 and u == 3 and pm_prev is not None:
                            emit_attnv(h - 1, pm_prev)
                    pe = pall.tile([128, 1024], BF16, tag="pe")
                    nc.scalar.activation(
                        pe[:], m[:], mybir.ActivationFunctionType.Exp)
                    pm.append(pe)
                pm_prev = pm

            emit_attnv(H - 1, pm_prev)

            for p in range(4):
                emit_pair_epilogue(p)

            # wo bias via ones-row matmul, close the accumulation group
            nc.tensor.matmul(psy[:], lhsT=ones_row[:],
                             rhs=bo_row[:], start=False, stop=True)
            y1 = persist.tile([128, D], F32, tag="y1")
            nc.vector.tensor_copy(y1[:], psy[:])

            # LN4 -> z (bf16), transpose, wp projection + bp
            z = persist.tile([128, D], BF16, tag="z")
            emit_ln(y1, z)
            tpz = psum.tile([128, 512], BF16, tag="wide", name="tpz")
            for bb in range(4):
                nc.tensor.transpose(tpz[:, bb * 128:(bb + 1) * 128],
                                    z[:, bb * 128:(bb + 1) * 128], eye[:])
            zT = persist.tile([128, 4, TSH], BF16, tag="zT")
            nc.vector.tensor_copy(
                zT[:], tpz[:].rearrange("p (b t) -> p b t", b=4))
            psy2 = psum.tile([128, 512], F32, tag="psy", bufs=1, name="psy2")
            for bb in range(4):
                nc.tensor.matmul(psy2[:], lhsT=zT[:, bb, :],
                                 rhs=w_sb["wp"][:, bb, :],
                                 start=(bb == 0), stop=False)
            nc.tensor.matmul(psy2[:], lhsT=ones_row[:],
                             rhs=bp_row[:], start=False, stop=True)
            yt = persist.tile([128, P], BF16, tag="yt")
            nc.vector.tensor_copy(yt[:], psy2[:])
            nc.sync.dma_start(y_d[:], yt[:])

    nc.compile()
    return nc


def _make_ind():
    ind = np.zeros((2, 128), np.float32)
    ind[0, 0:64] = 1.0
    ind[1, 64:128] = 1.0
    return ind


def _prep_host(inputs):
    """Fold LN weights/biases and 1/sqrt(dk) into projection weights; build
    per-core input maps."""
    f32 = np.float32
    bf16 = ml_dtypes.bfloat16
    me = np.ascontiguousarray(inputs["model_embed"], dtype=f32).reshape(BT, D)
    kin = np.asarray(inputs["context_embed_key"], dtype=f32)
    vin = np.asarray(inputs["context_embed_value"], dtype=f32)
    g = lambda n: np.asarray(inputs[n], dtype=f32)

    scale = 1.0 / np.sqrt(DK)
    wq_eff = (g("ln1_w")[:, None] * g("wq")) * scale
    bq_eff = (g("ln1_b") @ g("wq") + g("bq")) * scale
    wk_eff = g("ln2_w")[:, None] * g("wk")
    bk_eff = g("ln2_b") @ g("wk") + g("bk")
    wv_eff = g("ln3_w")[:, None] * g("wv")
    bv_eff = g("ln3_b") @ g("wv") + g("bv")
    wo_eff = g("wo")
    bo_eff = g("bo")
    wp_eff = g("ln4_w")[:, None] * g("wp")
    bp_eff = g("ln4_b") @ g("wp") + g("bp")

    # overflow guard for the zero-offset softmax: |logits| must stay << 87
    def smax(w):
        v = np.random.RandomState(0).randn(w.shape[1]).astype(f32)
        for _ in range(20):
            v = w.T @ (w @ v)
            v /= np.linalg.norm(v)
        return np.linalg.norm(w @ v)
    bound = ((np.sqrt(D) * smax(wq_eff) + np.linalg.norm(bq_eff))
             * (np.sqrt(D) * smax(wk_eff) + np.linalg.norm(bk_eff)))
    assert bound < 80.0, f"logit bound {bound} too large for exp without max"

    common = {
        "x": me.astype(bf16),
        "wq": wq_eff.astype(bf16), "wk": wk_eff.astype(bf16),
        "wv": wv_eff.astype(bf16), "wo": wo_eff.astype(bf16),
        "wp": wp_eff.astype(bf16),
        "bq": bq_eff, "bk": bk_eff, "bv": bv_eff,
        "bo": bo_eff.astype(bf16), "bp": bp_eff.astype(bf16),
        "ind": _make_ind().astype(bf16),
        "eye": np.eye(128, dtype=bf16),
    }
    in_maps = []
    for c in range(NCORES):
        ksh = kin[c * CS:(c + 1) * CS]             # [CS, U, D]
        ksh = np.ascontiguousarray(
            ksh.transpose(1, 0, 2).reshape(CUS, D))  # u-major rows
        vsh = np.ascontiguousarray(vin[c * CS:(c + 1) * CS])
        m = dict(common)
        m["kin"] = ksh.astype(bf16)
        m["vin"] = vsh.astype(bf16)
        in_maps.append(m)
    return in_maps


def kernel(**inputs) -> np.ndarray:
    if "nc" not in _CACHE:
        _CACHE["nc"] = build_nc()
    nc = _CACHE["nc"]
    in_maps = _prep_host(inputs)
    res = bass_utils.run_bass_kernel_spmd(
        nc, in_maps, core_ids=list(range(NCORES)))
    y = np.concatenate([res.results[c]["y"] for c in range(NCORES)], axis=0)
    return y.reshape(B, T, P).astype(np.float32)


if __name__ == "__main__":
    # quick smoke: random inputs of the right shapes
    print("building...")
    build_nc()
    print("ok")


# revision 32
# speedup vs baseline: 1.1055x; 1.1055x over previous
"""Trainium2 Bass kernel for nn_ColbertAdapter (ColBERT late-interaction adapter).

Strategy (8 NeuronCores, single SPMD launch):
  - Context index (C=2048 entries) is sharded 256/core; queries replicated.
  - Inputs uploaded bf16; LN weights + 1/sqrt(dk) folded into projection
    weights on host. LN normalize runs on the scalar engine (Identity with
    per-partition scale/bias), stats on vector.
  - PE p-state aware scheduling: the tensor engine only reaches 2.4 GHz
    after ~3us of gapless execution, so LN/transpose/projection emission is
    interleaved to keep the PE dense, score PSUM tiles are single-bank
    [128,512] with a 4-deep ring, and every PSUM consumer is split across
    DVE/ACT so the PE never waits.
  - Scores computed transposed S^T[cu, t]; MaxSim over U as bf16 chained
    maxes straight out of PSUM, one Exp per column group into bf16.
  - attn@v via PE with a ones-column augmenting v so the softmax denominator
    falls out of the same matmul. Softmax uses a fixed zero max-offset:
    LN output norm is exactly sqrt(D), so |logits| stays well below 87
    (checked on host via power iteration) and exp cannot overflow.
  - Per head-pair bf16 ReduceScatter merges partial numerators+denominators;
    a tiny warmup collective right after launch absorbs the first-collective
    setup latency; pair epilogues (divide -> wo accumulate) are pipelined
    under later heads' compute; only LN4 -> wp runs after the last RS.
"""

import os
import sys

try:
    import concourse  # noqa: F401
except ImportError:
    for p in ("/opt/trn_rl_repo", "/root/.axon_site/_ro/trn_rl_repo"):
        if os.path.isdir(p):
            sys.path.insert(0, p)
            break

import numpy as np
import ml_dtypes

import concourse.bass as bass
import concourse.mybir as mybir
from concourse import tile, bacc, bass_utils
from concourse.alu_op_type import AluOpType

BF16 = mybir.dt.bfloat16
F32 = mybir.dt.float32

NCORES = 8
B, T, C, U, D, P = 4, 256, 2048, 4, 512, 512
H = 8
DK = D // H
BT = B * T              # 1024 query tokens
CS = C // NCORES        # 256 contexts per core
CUS = CS * U            # 1024 key rows per core
TSH = BT // NCORES      # 128 tokens per core in the output shard
EPS = 1e-5

_CACHE = {}


def build_nc():
    nc = bacc.Bacc("TRN2", target_bir_lowering=False, debug=False,
                   num_devices=NCORES)

    # ---- DRAM I/O ----
    x_d = nc.dram_tensor("x", [BT, D], BF16, kind="ExternalInput").ap()
    kin_d = nc.dram_tensor("kin", [CUS, D], BF16, kind="ExternalInput").ap()
    vin_d = nc.dram_tensor("vin", [CS, D], BF16, kind="ExternalInput").ap()
    w_d = {
        n: nc.dram_tensor(n, [D, D], BF16, kind="ExternalInput").ap()
        for n in ("wq", "wk", "wv", "wo", "wp")
    }
    bq_d = nc.dram_tensor("bq", [D], F32, kind="ExternalInput").ap()
    bk_d = nc.dram_tensor("bk", [D], F32, kind="ExternalInput").ap()
    bv_d = nc.dram_tensor("bv", [D], F32, kind="ExternalInput").ap()
    bo_d = nc.dram_tensor("bo", [D], BF16, kind="ExternalInput").ap()
    ind_d = nc.dram_tensor("ind", [2, 128], BF16, kind="ExternalInput").ap()
    eye_d = nc.dram_tensor("eye", [128, 128], BF16, kind="ExternalInput").ap()
    bp_d = nc.dram_tensor("bp", [D], BF16, kind="ExternalInput").ap()
    y_d = nc.dram_tensor("y", [TSH, P], BF16, kind="ExternalOutput").ap()

    with tile.TileContext(nc) as tc:
        from contextlib import ExitStack
        ctx = ExitStack()
        with ctx:
            persist = ctx.enter_context(tc.tile_pool(name="persist", bufs=1))
            small = ctx.enter_context(tc.tile_pool(name="small", bufs=4))
            lnin = ctx.enter_context(tc.tile_pool(name="lnin", bufs=6))
            lnout = ctx.enter_context(tc.tile_pool(name="lnout", bufs=6))
            pall = ctx.enter_context(tc.tile_pool(name="pall", bufs=4))
            pmax = ctx.enter_context(tc.tile_pool(name="pmax", bufs=6))
            o65p = ctx.enter_context(tc.tile_pool(name="o65", bufs=4))
            psum = ctx.enter_context(
                tc.tile_pool(name="psum", bufs=3, space="PSUM"))
            dram = ctx.enter_context(
                tc.tile_pool(name="dram", bufs=1, space="DRAM"))

            # ---- small constants first (transposes need eye) ----
            eye = persist.tile([128, 128], BF16, tag="eye")
            nc.sync.dma_start(eye[:], eye_d)
            ind = persist.tile([2, 128], BF16, tag="ind")
            nc.sync.dma_start(ind[:], ind_d)

            # tiny warmup collective: pays the first-collective setup cost
            # while LN/projections run, so the real RS0 starts promptly
            warm_sb = persist.tile([8, 64], BF16, tag="warm")
            nc.vector.memset(warm_sb[:], 0.0)
            warm_in = dram.tile([8, 64], BF16, name="warm_in")
            warm_out = dram.tile([1, 64], BF16, name="warm_out")
            nc.sync.dma_start(warm_in[:], warm_sb[:])
            nc.gpsimd.collective_compute(
                "ReduceScatter", AluOpType.add,
                replica_groups=[list(range(NCORES))],
                ins=[warm_in.opt()], outs=[warm_out.opt()],
            )

            # ---- LayerNorm helper: stats on DVE, normalize on ACT ----
            def emit_ln(x_tile, out_tile):
                stats6 = small.tile([128, 6], F32, tag="bns")
                nc.vector.bn_stats(stats6[:], x_tile[:])
                mv = small.tile([128, 2], F32, tag="bna")
                nc.vector.bn_aggr(mv[:], stats6[:])
                veps = small.tile([128, 1], F32, tag="veps")
                nc.vector.tensor_scalar_add(veps[:], mv[:, 1:2], EPS)
                std = small.tile([128, 1], F32, tag="std")
                nc.scalar.sqrt(std[:], veps[:])
                rstd = small.tile([128, 1], F32, tag="rstd")
                nc.vector.reciprocal(rstd[:], std[:])
                negmr = small.tile([128, 1], F32, tag="negmr")
                nc.vector.tensor_scalar(
                    negmr[:], mv[:, 0:1], rstd[:], -1.0,
                    op0=AluOpType.mult, op1=AluOpType.mult)
                nc.scalar.activation(
                    out_tile[:], x_tile[:],
                    mybir.ActivationFunctionType.Identity,
                    bias=negmr[:, 0:1], scale=rstd[:, 0:1])

            evac_i = [0]

            def evac_engine():
                evac_i[0] += 1
                return nc.vector if evac_i[0] % 2 == 0 else nc.scalar

            # ---- LN + transpose staging ----
            def stage(src_ap, dstT, tiles):
                # dstT: [128, 4, n_tiles, 128] bf16 == srcLN^T blocks
                for i in tiles:
                    xt = lnin.tile([128, D], BF16, tag="lnin")
                    nc.sync.dma_start(xt[:], src_ap[i * 128:(i + 1) * 128, :])
                    lt = lnout.tile([128, D], BF16, tag="lnout")
                    emit_ln(xt, lt)
                    tp = psum.tile([128, 512], BF16, tag="wide")
                    for jb in range(4):
                        nc.tensor.transpose(
                            tp[:, jb * 128:(jb + 1) * 128],
                            lt[:, jb * 128:(jb + 1) * 128], eye[:])
                    eng = evac_engine()
                    if eng is nc.vector:
                        nc.vector.tensor_copy(
                            dstT[:, :, i, :],
                            tp[:].rearrange("p (b t) -> p b t", b=4))
                    else:
                        nc.scalar.copy(
                            dstT[:, :, i, :],
                            tp[:].rearrange("p (b t) -> p b t", b=4))

            def proj_chunk(dstT, wname, srcT, bias, tch):
                # dstT[:, jt, tch*512:(tch+1)*512] over all 4 jt blocks
                for jt in range(4):
                    ps = psum.tile([128, 512], F32, tag="wide",
                                   name=f"ps_{wname}_{jt}_{tch}")
                    for dt in range(4):
                        nc.tensor.matmul(
                            ps[:],
                            lhsT=w_sb[wname][:, dt,
                                             jt * 128:(jt + 1) * 128],
                            rhs=srcT[:, dt, tch * 4:(tch + 1) * 4, :],
                            start=(dt == 0), stop=(dt == 3))
                    eng = evac_engine()
                    dst = dstT[:, jt, tch * 512:(tch + 1) * 512]
                    if eng is nc.vector:
                        nc.vector.tensor_scalar_add(
                            dst, ps[:], bias[:, jt:jt + 1])
                    else:
                        nc.scalar.add(dst, ps[:], bias[:, jt:jt + 1])

            # weights / biases: issue DMAs interleaved with LN input DMAs so
            # the first projection's weights arrive just in time
            w_sb = {}
            for n in ("wq", "wk", "wv", "wo", "wp"):
                w_sb[n] = persist.tile([128, 4, D], BF16, tag=f"w_{n}",
                                       name=f"w_{n}")

            def load_w(n):
                nc.sync.dma_start(
                    w_sb[n][:], w_d[n].rearrange("(b p) j -> p b j", p=128))

            bq_sb = persist.tile([128, 4], F32, tag="bq")
            bk_sb = persist.tile([128, 4], F32, tag="bk")
            bv_sb = persist.tile([128, 4], F32, tag="bv")

            xnT = persist.tile([128, 4, 8, 128], BF16, tag="xnT")
            knT = persist.tile([128, 4, 8, 128], BF16, tag="knT")
            vnT = persist.tile([128, 4, 2, 128], BF16, tag="vnT")
            qT = persist.tile([128, 4, BT], BF16, tag="qT")
            kT = persist.tile([128, 4, CUS], BF16, tag="kT")

            stage(x_d, xnT, [0, 1, 2, 3])
            load_w("wq")
            nc.sync.dma_start(bq_sb[:], bq_d.rearrange("(b p) -> p b", p=128))
            stage(x_d, xnT, [4, 5, 6, 7])
            proj_chunk(qT, "wq", xnT, bq_sb, 0)
            stage(kin_d, knT, [0, 1, 2, 3])
            load_w("wk")
            nc.sync.dma_start(bk_sb[:], bk_d.rearrange("(b p) -> p b", p=128))
            proj_chunk(qT, "wq", xnT, bq_sb, 1)
            stage(kin_d, knT, [4, 5, 6, 7])
            proj_chunk(kT, "wk", knT, bk_sb, 0)
            stage(vin_d, vnT, [0, 1])
            load_w("wv")
            nc.sync.dma_start(bv_sb[:], bv_d.rearrange("(b p) -> p b", p=128))
            proj_chunk(kT, "wk", knT, bk_sb, 1)
            load_w("wo")
            load_w("wp")
            bo_row = persist.tile([1, D], BF16, tag="bo_row")
            nc.sync.dma_start(bo_row[:], bo_d.rearrange("(o d) -> o d", o=1))
            bp_row = persist.tile([1, D], BF16, tag="bp_row")
            nc.sync.dma_start(bp_row[:], bp_d.rearrange("(o d) -> o d", o=1))
            ones_row = persist.tile([1, 128], BF16, tag="ones_row")
            nc.vector.memset(ones_row[:], 1.0)

            # v[c, hd] with ones column per head -> v_sb[ct]: [128, 8, 65]
            # (bv is added after the attention average: attn weights sum to 1)
            v_sb = []
            for ct in range(2):
                vt = persist.tile([128, 8, 65], BF16, tag=f"v_sb{ct}",
                                  name=f"v_sb{ct}")
                ps = psum.tile([128, 512], F32, tag="wide")
                for dt in range(4):
                    nc.tensor.matmul(
                        ps[:],
                        lhsT=vnT[:, dt, ct, :],
                        rhs=w_sb["wv"][:, dt, :],
                        start=(dt == 0), stop=(dt == 3))
                nc.vector.tensor_copy(
                    vt[:, :, 0:64],
                    ps[:].rearrange("p (h e) -> p h e", h=8))
                nc.vector.memset(vt[:, :, 64:65], 1.0)
                v_sb.append(vt)

            # ---- per-head: scores^T -> U-max -> exp -> attn@v_aug ----
            bounce_ins = [
                dram.tile([NCORES, 130, TSH], BF16, name=f"bin{i}")
                for i in range(4)
            ]
            bounce_outs = [
                dram.tile([130, TSH], BF16, name=f"bout{i}")
                for i in range(4)
            ]
            bviews = [bo_.rearrange("(h j) t -> h j t", j=65)
                      for bo_ in bounce_outs]
            ob = persist.tile([128, 4, TSH], BF16, tag="ob")
            s_sb = persist.tile([2, 4, TSH], BF16, tag="s_sb")
            o_n = persist.tile([128, 4, TSH], BF16, tag="o_n")
            psy = psum.tile([128, 512], F32, tag="psy", bufs=1, name="psy")

            def emit_pair_epilogue(p):
                # readback merged o^T (+denominators) for our token shard
                nc.sync.dma_start(ob[0:64, p, :], bviews[p][0, 0:64, :])
                nc.sync.dma_start(ob[64:128, p, :], bviews[p][1, 0:64, :])
                nc.sync.dma_start(s_sb[:, p, :], bviews[p][:, 64, :])
                # broadcast the 2 denominator rows to the pair's 128 o^T rows
                psd = psum.tile([128, TSH], F32, tag="attnv", bufs=1,
                                name=f"psd{p}")
                nc.tensor.matmul(psd[:], lhsT=ind[:],
                                 rhs=s_sb[:, p, :],
                                 start=True, stop=True)
                rb = small.tile([128, TSH], F32, tag="rb")
                nc.vector.reciprocal(rb[:], psd[:])
                t = small.tile([128, TSH], F32, tag="odiv")
                nc.vector.tensor_mul(t[:], ob[:, p, :], rb[:])
                nc.vector.tensor_scalar_add(
                    o_n[:, p, :], t[:], bv_sb[:, p:p + 1])
                nc.tensor.matmul(psy[:], lhsT=o_n[:, p, :],
                                 rhs=w_sb["wo"][:, p, :],
                                 start=(p == 0), stop=False)

            def emit_attnv(h, pm):
                # attn@v for head h (software-pipelined one head behind the
                # score matmuls so the PE never waits on the exp chain)
                o65 = o65p.tile([65, 1024], BF16, tag="o65")
                for tch in range(2):
                    pso = psum.tile([65, 512], F32, tag="attnv", bufs=1,
                                    name=f"pso{h}_{tch}")
                    for c2 in range(2):
                        nc.tensor.matmul(
                            pso[:],
                            lhsT=v_sb[c2][:, h, :],
                            rhs=pm[c2][:, tch * 512:(tch + 1) * 512],
                            start=(c2 == 0), stop=(c2 == 1))
                    eng = nc.scalar if tch == 0 else nc.vector
                    if eng is nc.scalar:
                        nc.scalar.copy(
                            o65[:, tch * 512:(tch + 1) * 512], pso[:])
                    else:
                        nc.vector.tensor_copy(
                            o65[:, tch * 512:(tch + 1) * 512], pso[:])
                b_in = bounce_ins[h // 2]
                hh = h % 2
                nc.sync.dma_start(
                    b_in[:, hh * 65:(hh + 1) * 65, :].rearrange(
                        "s r t -> r s t"),
                    o65.rearrange("r (s t) -> r s t", s=NCORES))
                if h % 2 == 1:
                    nc.gpsimd.collective_compute(
                        "ReduceScatter", AluOpType.add,
                        replica_groups=[list(range(NCORES))],
                        ins=[bounce_ins[h // 2].rearrange("s r t -> (s r) t")],
                        outs=[bounce_outs[h // 2].opt()],
                    )

            pm_prev = None
            for h in range(H):
                hp = (h % 2) * 64
                jt = h // 2
                pm = []
                for c2 in range(2):
                    # 5-copy/3-mixed split: ACT evacuates most PSUM tiles to
                    # bf16; DVE chains maxes (one PSUM operand max each)
                    ncopy = 3 if c2 == 0 else 2
                    m = None
                    pss = []
                    for u in range(4):
                        r = 2 * u + c2
                        ps = psum.tile([128, 1024], F32, tag="wide",
                                       name=f"ps_h{h}_r{r}")
                        for tch in range(2):
                            nc.tensor.matmul(
                                ps[:, tch * 512:(tch + 1) * 512],
                                lhsT=kT[hp:hp + 64, jt,
                                        r * 128:(r + 1) * 128],
                                rhs=qT[hp:hp + 64, jt,
                                       tch * 512:(tch + 1) * 512],
                                start=True, stop=True)
                        last = (u == 3)
                        dst = None
                        if u < ncopy:
                            s = pmax.tile([128, 1024], BF16, tag="pm")
                            nc.scalar.copy(s[:], ps[:])
                            if m is None:
                                m = s
                            else:
                                m2 = (pall.tile([128, 1024], BF16, tag="pmf",
                                                name=f"pmf{h}_{c2}")
                                      if last else
                                      pmax.tile([128, 1024], BF16, tag="pm"))
                                nc.vector.tensor_max(m2[:], s[:], m[:])
                                m = m2
                        else:
                            m2 = (pall.tile([128, 1024], BF16, tag="pmf",
                                            name=f"pmf{h}_{c2}")
                                  if last else
                                  pmax.tile([128, 1024], BF16, tag="pm"))
                            nc.vector.tensor_max(m2[:], ps[:], m[:])
                            m = m2
                        # software-pipelined attnv of the previous head goes
                        # right after the first column group's matmuls
                        if c2 == 0 and u == 3 and pm_prev is not None:
                            emit_attnv(h - 1, pm_prev)
                    pe = pall.tile([128, 1024], BF16, tag="pe")
                    nc.scalar.activation(
                        pe[:], m[:], mybir.ActivationFunctionType.Exp)
                    pm.append(pe)
                pm_prev = pm

            emit_attnv(H - 1, pm_prev)

            for p in range(4):
                emit_pair_epilogue(p)

            # wo bias via ones-row matmul, close the accumulation group
            nc.tensor.matmul(psy[:], lhsT=ones_row[:],
                             rhs=bo_row[:], start=False, stop=True)
            y1 = persist.tile([128, D], F32, tag="y1")
            nc.vector.tensor_copy(y1[:], psy[:])

            # LN4 -> z (bf16), transpose, wp projection + bp
            z = persist.tile([128, D], BF16, tag="z")
            emit_ln(y1, z)
            tpz = psum.tile([128, 512], BF16, tag="wide", name="tpz")
            for bb in range(4):
                nc.tensor.transpose(tpz[:, bb * 128:(bb + 1) * 128],
                                    z[:, bb * 128:(bb + 1) * 128], eye[:])
            zT = persist.tile([128, 4, TSH], BF16, tag="zT")
            nc.vector.tensor_copy(
                zT[:], tpz[:].rearrange("p (b t) -> p b t", b=4))
            psy2 = psum.tile([128, 512], F32, tag="psy", bufs=1, name="psy2")
            for bb in range(4):
                nc.tensor.matmul(psy2[:], lhsT=zT[:, bb, :],
                                 rhs=w_sb["wp"][:, bb, :],
                                 start=(bb == 0), stop=False)
            nc.tensor.matmul(psy2[:], lhsT=ones_row[:],
                             rhs=bp_row[:], start=False, stop=True)
            yt = persist.tile([128, P], BF16, tag="yt")
            nc.vector.tensor_copy(yt[:], psy2[:])
            nc.sync.dma_start(y_d[:], yt[:])

    nc.compile()
    return nc


def _make_ind():
    ind = np.zeros((2, 128), np.float32)
    ind[0, 0:64] = 1.0
    ind[1, 64:128] = 1.0
    return ind


def _prep_host(inputs):
    """Fold LN weights/biases and 1/sqrt(dk) into projection weights; build
    per-core input maps."""
    f32 = np.float32
    bf16 = ml_dtypes.bfloat16
    me = np.ascontiguousarray(inputs["model_embed"], dtype=f32).reshape(BT, D)
    kin = np.asarray(inputs["context_embed_key"], dtype=f32)
    vin = np.asarray(inputs["context_embed_value"], dtype=f32)
    g = lambda n: np.asarray(inputs[n], dtype=f32)

    scale = 1.0 / np.sqrt(DK)
    wq_eff = (g("ln1_w")[:, None] * g("wq")) * scale
    bq_eff = (g("ln1_b") @ g("wq") + g("bq")) * scale
    wk_eff = g("ln2_w")[:, None] * g("wk")
    bk_eff = g("ln2_b") @ g("wk") + g("bk")
    wv_eff = g("ln3_w")[:, None] * g("wv")
    bv_eff = g("ln3_b") @ g("wv") + g("bv")
    wo_eff = g("wo")
    bo_eff = g("bo")
    wp_eff = g("ln4_w")[:, None] * g("wp")
    bp_eff = g("ln4_b") @ g("wp") + g("bp")

    # overflow guard for the zero-offset softmax: |logits| must stay << 87
    def smax(w):
        v = np.random.RandomState(0).randn(w.shape[1]).astype(f32)
        for _ in range(20):
            v = w.T @ (w @ v)
            v /= np.linalg.norm(v)
        return np.linalg.norm(w @ v)
    bound = ((np.sqrt(D) * smax(wq_eff) + np.linalg.norm(bq_eff))
             * (np.sqrt(D) * smax(wk_eff) + np.linalg.norm(bk_eff)))
    assert bound < 80.0, f"logit bound {bound} too large for exp without max"

    common = {
        "x": me.astype(bf16),
        "wq": wq_eff.astype(bf16), "wk": wk_eff.astype(bf16),
        "wv": wv_eff.astype(bf16), "wo": wo_eff.astype(bf16),
        "wp": wp_eff.astype(bf16),
        "bq": bq_eff, "bk": bk_eff, "bv": bv_eff,
        "bo": bo_eff.astype(bf16), "bp": bp_eff.astype(bf16),
        "ind": _make_ind().astype(bf16),
        "eye": np.eye(128, dtype=bf16),
    }
    in_maps = []
    for c in range(NCORES):
        ksh = kin[c * CS:(c + 1) * CS]             # [CS, U, D]
        ksh = np.ascontiguousarray(
            ksh.transpose(1, 0, 2).reshape(CUS, D))  # u-major rows
        vsh = np.ascontiguousarray(vin[c * CS:(c + 1) * CS])
        m = dict(common)
        m["kin"] = ksh.astype(bf16)
        m["vin"] = vsh.astype(bf16)
        in_maps.append(m)
    return in_maps


def kernel(**inputs) -> np.ndarray:
    if "nc" not in _CACHE:
        _CACHE["nc"] = build_nc()
    nc = _CACHE["nc"]
    in_maps = _prep_host(inputs)
    res = bass_utils.run_bass_kernel_spmd(
        nc, in_maps, core_ids=list(range(NCORES)))
    y = np.concatenate([res.results[c]["y"] for c in range(NCORES)], axis=0)
    return y.reshape(B, T, P).astype(np.float32)


if __name__ == "__main__":
    # quick smoke: random inputs of the right shapes
    print("building...")
    build_nc()
    print("ok")
